# revision 38
# baseline (speedup 1.0000x reference)
"""Causal single-head attention (B=4, T=4096, E=1024, H=64) on 8 trn2 cores.

reference:
    q,k,v = x@Wq, x@Wk, x@Wv          # per batch
    s = q @ k.T  (causal masked)
    out = (softmax(s) / sqrt(64)) @ v

Sharding: core c = 2*b + s handles batch b; queries are striped by 128-row
blocks (core s owns global q-blocks j with j % 2 == s).  The host rolls each
core's copy of X down by 128*s rows, which makes every core's program
identical: own queries are the local-EVEN 128-blocks, and the causal
structure relative to local coordinates is core-independent.  The rolled-away
first block re-enters as local block 31 ("wrap" block); its mask is all-ones
for s=1 (those keys precede everything) and all-zeros for s=0 (handled by
the normal diagonal path instead) — shipped as per-core mask data.

Per core: stream X (fp32r) -> PE-transpose -> fp32r matmuls for per-group
K^T/V/Q^T tiles; attention per 512-query macro-tile: S^T = K_blk @ Q^T in
PSUM, exp on ACT (bias -30; cancels in the softmax ratio), causal 0/1 mask
multiplies, P^T @ V_aug accumulated in PSUM (ones-column of 8.0 yields the
softmax denominator * 8, folding the /sqrt(64)), transpose back, divide,
write own output rows.  Projection group 7 is processed first (the wrap
block lives there) and attention macro-tiles are interleaved between
projection groups so the PE never idles long enough to lose its clock boost.
"""
import sys

if "/opt/trn_rl_repo" not in sys.path:
    sys.path.insert(0, "/opt/trn_rl_repo")

import numpy as np

import concourse.bass as bass
import concourse.tile as tile
from concourse import mybir
from concourse.vector_clock import ScopedClock, VectorClock

B = 4
T = 4096
E = 1024
H = 64
HA = H + 1
NCORES = 8
NG = 8          # groups of 512 rows
GR = 512        # rows per group
NEC = 8         # 128-wide chunks of E
NMAC = 4        # 512-query attention macro-tiles per core
NQB = 32        # 128-row key blocks
EXP_BIAS = -30.0009765625

F32 = mybir.dt.float32
F32R = mybir.dt.float32r

# const-image layout (columns of the single [128, CW] const DMA)
C_ID = 0                    # identity [128, 128]
C_WK = C_ID + 128           # wk image [128, 8*64]
C_WQ = C_WK + 512
C_WV = C_WQ + 512
C_EIGHT = C_WV + 512        # 8.0 x 32
C_BIAS = C_EIGHT + 32       # exp bias column
C_WKV = C_BIAS + 1          # fused [Wk|Wv] image [128, 8*128]
CW = C_WKV + 8 * 128

MW = 8 * 512                # masks image [128, 4096]: d0..d6, wrap

# mask sub-regions (piecewise; skip all-ones parts) for the 4 masked
# 2048-col chunks == kblocks [d0..d3] and [d4,d5,d6,wrap]
MASK_REGIONS = [
    [(0, 128), (512, 128), (1024, 256), (1536, 256)],
    [(0, 384), (512, 384), (1024, 512), (1536, 512)],
]


# Enable walrus LDWEIGHTS dedupe (the repo default disables it).
import concourse.bass_utils as _bu

if not getattr(_bu, "_ldw_opt_patched", False):
    _orig_bvo = _bu.bir_verify_and_optimise

    def _bvo_ldw(*args, **kwargs):
        import concourse.bass_utils as bu
        orig_run = bu.run_command

        def run_patched(argv, **kw):
            argv = [
                a.replace("--enable-ldw-opt=false", "--enable-ldw-opt=false")
                for a in argv
            ]
            return orig_run(argv, **kw)

        bu.run_command = run_patched
        try:
            return _orig_bvo(*args, **kwargs)
        finally:
            bu.run_command = orig_run

    _bu.bir_verify_and_optimise = _bvo_ldw
    _bu._ldw_opt_patched = True

_cache = {}



# ---------------------------------------------------------------------------
# Walrus in this container encodes at most ONE sync wait per instruction, and
# the Tile kernel-tail drain normally carries many.  Put the tail waits on
# dummy scalar-engine copies (one wait each); all body multi-waits are split
# by split_multi_waits() below.  All DMAs go through gpsimd so DMA waits
# never land on SP instructions.
# ---------------------------------------------------------------------------
class SplitDrainTileContext(tile.TileContext):
    def _drain_and_barrier(self, tick_clock, wait_clock):
        dummy = self.nc._tail_drain_dummy_ap
        gc = tick_clock.global_clock
        n = len(gc)
        for p in [i for i in range(n) if gc[i] > 0]:
            vec = [0] * n
            vec[p] = gc[p]
            carrier = self.nc.scalar.copy(dummy[:, :], dummy[:, :])
            wait_clock.add_sem_waits(
                carrier.ins, ScopedClock({None: VectorClock(vec)})
            )
        self.nc.sync.drain()

        self.nc.all_engine_barrier()
        assert self.sems is not None
        popped = self.nc._tile_sem_poison_stack.pop()
        assert popped is self._sem_poison
        self.nc.clear_and_free_semaphores(list(self.sems.allocated().values()))
        self.nc.all_engine_barrier()


def split_multi_waits(nc):
    """Move extra waits of any multi-wait instruction onto same-engine NOPs
    placed immediately before it (engine program order preserves semantics)."""
    for f in nc.m.functions:
        for bb in f.blocks:
            insts = bb.instructions
            i = 0
            while i < len(insts):
                inst = insts[i]
                si = inst.sync_info
                if si is not None and si.on_wait and len(si.on_wait) > 1:
                    waits = list(si.on_wait)
                    for w in waits[:-1]:
                        nop = mybir.InstNoOp(
                            name=f"I-{nc.next_id()}", ins=[], outs=[]
                        )
                        nop.engine = inst.engine
                        nop.sync_info = mybir.SyncInfo(on_wait=[w], on_update=[])
                        nc.register_instruction(nop)
                        insts.insert(i, nop)
                        i += 1
                    inst.sync_info = mybir.SyncInfo(
                        on_wait=[waits[-1]], on_update=list(si.on_update)
                    )
                i += 1


def build_kernel():
    nc = bass.Bass("TRN2", target_bir_lowering=False, debug=False)
    nc._tail_drain_dummy_ap = nc.alloc_sbuf_tensor(
        "tail_drain_dummy", [1, 1], F32
    ).ap()

    x = nc.dram_tensor("x", [T, E], F32, kind="ExternalInput").ap()
    cst = nc.dram_tensor("cst", [128, CW], F32, kind="ExternalInput").ap()
    masks = nc.dram_tensor("masks", [128, MW], F32, kind="ExternalInput").ap()
    out = nc.dram_tensor("out", [T // 2, H], F32, kind="ExternalOutput").ap()

    with SplitDrainTileContext(nc) as tc:
        _build_body(nc, tc, x, cst, masks, out)
    split_multi_waits(nc)
    return nc


def _build_body(nc, tc, x, cst, masks, out):
    from contextlib import ExitStack

    ctx = ExitStack()
    with ctx:
        const_pool = ctx.enter_context(tc.tile_pool(name="const", bufs=1))
        persist = ctx.enter_context(tc.tile_pool(name="persist", bufs=1))
        xg_pool = ctx.enter_context(tc.tile_pool(name="xg", bufs=12))
        xt_pool = ctx.enter_context(tc.tile_pool(name="xt", bufs=16))
        exps_pool = ctx.enter_context(tc.tile_pool(name="exps", bufs=3))
        small_pool = ctx.enter_context(tc.tile_pool(name="small", bufs=4))

        # ---- constants; identity + first X tiles first so the PE can start
        id_sbr = const_pool.tile([128, 128], F32R)
        nc.gpsimd.dma_start(id_sbr[:], cst[:, C_ID : C_ID + 128])

        xgs = {}
        def load_group(g):
            tiles = []
            for a in range(4):
                xga = xg_pool.tile([128, E], F32R, tag="xga")
                nc.gpsimd.dma_start(
                    xga[:], x[g * GR + a * 128 : g * GR + (a + 1) * 128, :]
                )
                tiles.append(xga)
            xgs[g] = tiles

        load_group(0)
        cst_sb = const_pool.tile([128, CW], F32)
        w_sb = const_pool.tile([128, 3 * 512], F32R)  # wk|wq|wv images
        wkv_sb = const_pool.tile([128, NEC * 128], F32R)  # fused [Wk|Wv]
        nc.gpsimd.dma_start(cst_sb[:], cst[:])
        nc.gpsimd.dma_start(w_sb[:], cst[:, C_WK : C_WK + 3 * 512])
        nc.gpsimd.dma_start(wkv_sb[:], cst[:, C_WKV : C_WKV + NEC * 128])
        load_group(1)
        load_group(2)
        masks_sb = const_pool.tile([128, MW], F32)
        nc.gpsimd.dma_start(masks_sb[:], masks[:])

        id_sb = cst_sb[:, C_ID : C_ID + 128]
        expbias = cst_sb[:, C_BIAS : C_BIAS + 1]

        def wchunk(base, ec):  # [128, 64] fp32r weight chunk
            return w_sb[:, base + ec * H : base + (ec + 1) * H]

        def wkv(ec):  # [128, 128] fused [Wk | Wv] stationary
            return wkv_sb[:, ec * 128 : (ec + 1) * 128]

        # ---- persistent intermediates
        # kt_all: paired layout for PE row-tiling — K^T block i lives at
        # rows (i%2)*64..+64, cols (i//2)*128..+128
        kt_all = persist.tile([128, T // 2], F32R)
        # qt_all: rows 0:64 = Q^T (own queries), rows 64:128 = duplicate
        qt_all = persist.tile([128, T // 2], F32R)
        vaug = persist.tile([128, NQB * HA], F32R)   # V blocks + denom col
        outstage = persist.tile([128, 16 * H], F32)

        # denominator column = 8.0 (folds the /sqrt(64))
        nc.vector.tensor_copy(
            vaug[:].rearrange("p (i c) -> p i c", c=HA)[:, :, H],
            cst_sb[:, C_EIGHT : C_EIGHT + 32],
        )

        # ================= phase P: projections =================
        pp = ExitStack()
        with pp:
            xt_ps_pool = pp.enter_context(
                tc.tile_pool(name="xt_ps", bufs=4, space="PSUM")
            )
            kt_ps_pool = pp.enter_context(
                tc.tile_pool(name="kt_ps", bufs=1, space="PSUM")
            )
            qt_ps_pool = pp.enter_context(
                tc.tile_pool(name="qt_ps", bufs=1, space="PSUM")
            )
            vtr_ps_pool = pp.enter_context(
                tc.tile_pool(name="vtr_ps", bufs=2, space="PSUM")
            )

            for g in range(NG):
                if g + 3 < NG:
                    load_group(g + 3)
                xga = xgs.pop(g)

                # transpose X group: xt[ec] = X[group rows, ec-chunk].T
                xts = []
                for ec in range(NEC):
                    xt_ps = xt_ps_pool.tile([128, GR], F32R)
                    for a in range(4):
                        nc.tensor.transpose(
                            xt_ps[:, a * 128 : (a + 1) * 128],
                            xga[a][:, ec * 128 : (ec + 1) * 128],
                            id_sbr[:],
                        )
                    xt_sb = xt_pool.tile([128, GR], F32R)
                    eng = nc.vector.tensor_copy if ec % 2 == 0 else nc.scalar.copy
                    eng(xt_sb[:], xt_ps[:])
                    xts.append(xt_sb)

                # fused [Wk|Wv] chain: rows 0:64 = K^T, rows 64:128 = V^T
                kv_ps = kt_ps_pool.tile([128, GR], F32, tag="kv_ps")
                for ec in range(NEC):
                    nc.tensor.matmul(
                        kv_ps[:], wkv(ec), xts[ec][:],
                        start=(ec == 0), stop=(ec == NEC - 1),
                    )
                evens = kv_ps[0:64, :].rearrange("p (a r m) -> p r a m", a=2, r=2)
                nc.vector.tensor_copy(
                    kt_all[0:64, 2 * g * 128 : (2 * g + 2) * 128].rearrange(
                        "p (a m) -> p a m", a=2
                    ),
                    evens[:, 0, :, :],
                )
                ktodd = small_pool.tile([64, 256], F32R, tag="ktodd")
                nc.vector.tensor_copy(
                    ktodd[:].rearrange("p (a m) -> p a m", a=2),
                    evens[:, 1, :, :],
                )
                nc.gpsimd.dma_start(
                    kt_all[64:128, 2 * g * 128 : (2 * g + 2) * 128], ktodd[:]
                )

                # V^T (rows 64:128) -> V natural blocks via PE transpose
                vt_sb = small_pool.tile([128, GR], F32, tag="vt_sb")
                nc.scalar.copy(vt_sb[64:128, :], kv_ps[64:128, :])
                for a in range(4):
                    vtr_ps = vtr_ps_pool.tile([128, H], F32)
                    nc.tensor.transpose(
                        vtr_ps[:],
                        vt_sb[64:128, a * 128 : (a + 1) * 128],
                        id_sb[64:128, 64:128],
                    )
                    i = 4 * g + a
                    nc.vector.tensor_copy(
                        vaug[:, i * HA : i * HA + H], vtr_ps[:]
                    )

                # Q^T chain: own queries are local blocks 4g, 4g+2
                qt_ps = qt_ps_pool.tile([64, 256], F32)
                for ec in range(NEC):
                    rhs = xts[ec][:].rearrange(
                        "p (a r m) -> p r a m", a=2, r=2
                    )[:, 0, :, :]
                    nc.tensor.matmul(
                        qt_ps[:].rearrange("p (a m) -> p a m", a=2),
                        wchunk(512, ec),
                        rhs,
                        start=(ec == 0), stop=(ec == NEC - 1),
                    )
                nc.vector.tensor_copy(
                    qt_all[0:64, g * 256 : (g + 1) * 256], qt_ps[:]
                )
                if g % 2 == 1:  # duplicate finished 512-col pair to rows 64+
                    jj = g // 2
                    nc.gpsimd.dma_start(
                        qt_all[64:128, jj * 512 : (jj + 1) * 512],
                        qt_all[0:64, jj * 512 : (jj + 1) * 512],
                    )

        # ================= phase A: attention =================
        CH = 3  # kblocks per PSUM chunk (3 banks; double-buffered)
        pa = ExitStack()
        with pa:
            s_ps_pool = pa.enter_context(
                tc.tile_pool(name="s_ps", bufs=2, space="PSUM")
            )
            av_ps_pool = pa.enter_context(
                tc.tile_pool(name="av_ps", bufs=1, space="PSUM")
            )
            ot_ps_pool = pa.enter_context(
                tc.tile_pool(name="ot_ps", bufs=1, space="PSUM")
            )

            for jj in range(NMAC):
                # key blocks: 8jj full, then d=0..6 (local 8jj..8jj+6), wrap
                kblocks = list(range(8 * jj + 7)) + [31]
                nk = len(kblocks)  # 8jj+8
                qs = qt_all[:, jj * 512 : (jj + 1) * 512]

                av_ps = av_ps_pool.tile([HA, 512], F32)
                state = {"first": True}
                pend = None

                def issue_av(pend_, stop):
                    pexps, pblocks = pend_
                    for ci, i in enumerate(pblocks):
                        nc.tensor.matmul(
                            av_ps[:],
                            vaug[:, i * HA : (i + 1) * HA],
                            pexps[:, ci * 512 : (ci + 1) * 512],
                            start=state["first"],
                            stop=stop and (ci == len(pblocks) - 1),
                        )
                        state["first"] = False

                chunks = [kblocks[c : c + CH] for c in range(0, nk, CH)]
                for c, blocks in enumerate(chunks):
                    n = len(blocks) * 512
                    s_ps = s_ps_pool.tile([128, CH * 512], F32)
                    for ci, i in enumerate(blocks):
                        rg = (i % 2) * 64  # paired row group
                        nc.tensor.matmul(
                            s_ps[:, ci * 512 : (ci + 1) * 512],
                            kt_all[rg : rg + 64, (i // 2) * 128 : (i // 2 + 1) * 128],
                            qs[rg : rg + 64, :],
                            start=True,
                            stop=True,
                        )
                    exps = exps_pool.tile([128, CH * 512], F32R)
                    nc.scalar.activation(
                        exps[:, 0:n], s_ps[:, 0:n],
                        mybir.ActivationFunctionType.Exp,
                        bias=expbias,
                    )
                    # causal masks (0/1 multiplies on the sub-regions that
                    # are not all-ones)
                    for ci, i in enumerate(blocks):
                        d = i - 8 * jj
                        if i == 31:
                            w, mbase = 512, 7 * 512      # wrap block
                        elif 0 <= d <= 6:
                            w, mbase = (d // 2 + 1) * 128, d * 512
                        else:
                            continue
                        nc.vector.tensor_mul(
                            exps[:, ci * 512 : ci * 512 + w],
                            exps[:, ci * 512 : ci * 512 + w],
                            masks_sb[:, mbase : mbase + w],
                        )
                    if pend is not None:
                        issue_av(pend, stop=False)
                    pend = (exps, blocks)
                issue_av(pend, stop=True)

                # out^T [65, 512] -> transpose -> divide -> stage -> DMA out
                avs = small_pool.tile([HA, 512], F32, tag="avs")
                nc.scalar.copy(avs[:], av_ps[:])
                for t in range(4):
                    ot_ps = ot_ps_pool.tile([128, HA], F32)
                    nc.tensor.transpose(
                        ot_ps[:],
                        avs[:, t * 128 : (t + 1) * 128],
                        id_sb[0:HA, 0:HA],
                    )
                    rcp = small_pool.tile([128, 1], F32, tag="rcp")
                    nc.vector.reciprocal(rcp[:], ot_ps[:, H : H + 1])
                    q128 = 4 * jj + t
                    nc.vector.tensor_scalar_mul(
                        outstage[:, q128 * H : (q128 + 1) * H],
                        ot_ps[:, 0:H],
                        rcp[:],
                    )
                nc.gpsimd.dma_start(
                    out[jj * 512 : (jj + 1) * 512, :].rearrange(
                        "(b p) h -> p b h", p=128
                    ),
                    outstage[:, jj * 4 * H : (jj + 1) * 4 * H].rearrange(
                        "p (b h) -> p b h", h=H
                    ),
                )


def _host_inputs(input, Wq, Wk, Wv):
    """Build the 8 per-core input maps from the full problem inputs."""
    triu = np.triu(np.ones((128, 128), dtype=np.float32))
    ones = np.ones((128, 128), dtype=np.float32)
    zeros = np.zeros((128, 128), dtype=np.float32)

    def wimg(w):
        return (
            np.asarray(w, dtype=np.float32)
            .reshape(NEC, 128, H)
            .transpose(1, 0, 2)
            .reshape(128, NEC * H)
        )

    cst = np.empty((128, CW), dtype=np.float32)
    cst[:, C_ID : C_ID + 128] = np.eye(128, dtype=np.float32)
    cst[:, C_WK : C_WK + 512] = wimg(Wk)
    cst[:, C_WQ : C_WQ + 512] = wimg(Wq)
    cst[:, C_WV : C_WV + 512] = wimg(Wv)
    cst[:, C_EIGHT : C_EIGHT + 32] = 8.0
    cst[:, C_BIAS] = EXP_BIAS
    wk_i, wv_i = wimg(Wk), wimg(Wv)
    for ec in range(NEC):
        cst[:, C_WKV + ec * 128 : C_WKV + ec * 128 + 64] = wk_i[
            :, ec * 64 : (ec + 1) * 64
        ]
        cst[:, C_WKV + ec * 128 + 64 : C_WKV + (ec + 1) * 128] = wv_i[
            :, ec * 64 : (ec + 1) * 64
        ]

    in_maps = []
    for c in range(NCORES):
        b, s = divmod(c, 2)
        xb = np.asarray(input[b])
        x_rot = np.roll(xb, -128 * s, axis=0) if s else xb
        wrap = ones if s == 1 else zeros
        m = np.empty((128, MW), dtype=np.float32)
        for d in range(7):  # mask_d: 4 col-blocks vs rel = d - 2m
            for mcol in range(4):
                rel = d - 2 * mcol
                blk = ones if rel < 0 else (triu if rel == 0 else zeros)
                m[:, d * 512 + mcol * 128 : d * 512 + (mcol + 1) * 128] = blk
        m[:, 7 * 512 :] = np.tile(wrap, (1, 4))
        in_maps.append(
            {
                "x": np.ascontiguousarray(x_rot, dtype=np.float32),
                "cst": cst,
                "masks": m,
            }
        )
    return in_maps


def _assemble(results):
    """Scatter per-core striped outputs back to [B, T, H]."""
    out = np.empty((B, T, H), dtype=np.float32)
    for c in range(NCORES):
        b, s = divmod(c, 2)
        o = results[c]["out"].reshape(16, 128, H)  # own blocks, in order
        view = out[b].reshape(32, 128, H)
        view[s::2] = o
    return out


def kernel(input, Wq, Wk, Wv):
    from concourse.bass_utils import run_bass_kernel_spmd

    if "nc" not in _cache:
        _cache["nc"] = build_kernel()
    nc = _cache["nc"]
    in_maps = _host_inputs(input, Wq, Wk, Wv)
    res = run_bass_kernel_spmd(nc, in_maps, core_ids=list(range(NCORES)))
    return _assemble(res.results)


# revision 39
# speedup vs baseline: 1.0141x; 1.0141x over previous
"""Causal single-head attention (B=4, T=4096, E=1024, H=64) on 8 trn2 cores.

reference:
    q,k,v = x@Wq, x@Wk, x@Wv          # per batch
    s = q @ k.T  (causal masked)
    out = (softmax(s) / sqrt(64)) @ v

Sharding: core c = 2*b + s handles batch b; queries are striped by 128-row
blocks (core s owns global q-blocks j with j % 2 == s).  The host rolls each
core's copy of X down by 128*s rows, which makes every core's program
identical: own queries are the local-EVEN 128-blocks, and the causal
structure relative to local coordinates is core-independent.  The rolled-away
first block re-enters as local block 31 ("wrap" block); its mask is all-ones
for s=1 (those keys precede everything) and all-zeros for s=0 (handled by
the normal diagonal path instead) — shipped as per-core mask data.

Per core: stream X (fp32r) -> PE-transpose -> fp32r matmuls for per-group
K^T/V/Q^T tiles; attention per 512-query macro-tile: S^T = K_blk @ Q^T in
PSUM, exp on ACT (bias -30; cancels in the softmax ratio), causal 0/1 mask
multiplies, P^T @ V_aug accumulated in PSUM (ones-column of 8.0 yields the
softmax denominator * 8, folding the /sqrt(64)), transpose back, divide,
write own output rows.  Projection group 7 is processed first (the wrap
block lives there) and attention macro-tiles are interleaved between
projection groups so the PE never idles long enough to lose its clock boost.
"""
import sys

if "/opt/trn_rl_repo" not in sys.path:
    sys.path.insert(0, "/opt/trn_rl_repo")

import numpy as np

import concourse.bass as bass
import concourse.tile as tile
from concourse import mybir
from concourse.vector_clock import ScopedClock, VectorClock

B = 4
T = 4096
E = 1024
H = 64
HA = H + 1
NCORES = 8
NG = 8          # groups of 512 rows
GR = 512        # rows per group
NEC = 8         # 128-wide chunks of E
NMAC = 4        # 512-query attention macro-tiles per core
NQB = 32        # 128-row key blocks
EXP_BIAS = -30.0009765625

F32 = mybir.dt.float32
F32R = mybir.dt.float32r

# const-image layout (columns of the single [128, CW] const DMA)
C_ID = 0                    # identity [128, 128]
C_WK = C_ID + 128           # wk image [128, 8*64]
C_WQ = C_WK + 512
C_WV = C_WQ + 512
C_EIGHT = C_WV + 512        # 8.0 x 32
C_BIAS = C_EIGHT + 32       # exp bias column
C_WKV = C_BIAS + 1          # fused [Wk|Wv] image [128, 8*128]
CW = C_WKV + 8 * 128

MW = 8 * 512                # masks image [128, 4096]: d0..d6, wrap

# mask sub-regions (piecewise; skip all-ones parts) for the 4 masked
# 2048-col chunks == kblocks [d0..d3] and [d4,d5,d6,wrap]
MASK_REGIONS = [
    [(0, 128), (512, 128), (1024, 256), (1536, 256)],
    [(0, 384), (512, 384), (1024, 512), (1536, 512)],
]


_cache = {}



# ---------------------------------------------------------------------------
# Walrus in this container encodes at most ONE sync wait per instruction, and
# the Tile kernel-tail drain normally carries many.  Put the tail waits on
# dummy scalar-engine copies (one wait each); all body multi-waits are split
# by split_multi_waits() below.  All DMAs go through gpsimd so DMA waits
# never land on SP instructions.
# ---------------------------------------------------------------------------
class SplitDrainTileContext(tile.TileContext):
    def _drain_and_barrier(self, tick_clock, wait_clock):
        dummy = self.nc._tail_drain_dummy_ap
        gc = tick_clock.global_clock
        n = len(gc)
        for p in [i for i in range(n) if gc[i] > 0]:
            vec = [0] * n
            vec[p] = gc[p]
            carrier = self.nc.scalar.copy(dummy[:, :], dummy[:, :])
            wait_clock.add_sem_waits(
                carrier.ins, ScopedClock({None: VectorClock(vec)})
            )
        self.nc.sync.drain()

        self.nc.all_engine_barrier()
        assert self.sems is not None
        popped = self.nc._tile_sem_poison_stack.pop()
        assert popped is self._sem_poison
        self.nc.clear_and_free_semaphores(list(self.sems.allocated().values()))
        self.nc.all_engine_barrier()


def split_multi_waits(nc):
    """Move extra waits of any multi-wait instruction onto same-engine NOPs
    placed immediately before it (engine program order preserves semantics)."""
    for f in nc.m.functions:
        for bb in f.blocks:
            insts = bb.instructions
            i = 0
            while i < len(insts):
                inst = insts[i]
                si = inst.sync_info
                if si is not None and si.on_wait and len(si.on_wait) > 1:
                    waits = list(si.on_wait)
                    for w in waits[:-1]:
                        nop = mybir.InstNoOp(
                            name=f"I-{nc.next_id()}", ins=[], outs=[]
                        )
                        nop.engine = inst.engine
                        nop.sync_info = mybir.SyncInfo(on_wait=[w], on_update=[])
                        nc.register_instruction(nop)
                        insts.insert(i, nop)
                        i += 1
                    inst.sync_info = mybir.SyncInfo(
                        on_wait=[waits[-1]], on_update=list(si.on_update)
                    )
                i += 1


def build_kernel():
    nc = bass.Bass("TRN2", target_bir_lowering=False, debug=False)
    nc._tail_drain_dummy_ap = nc.alloc_sbuf_tensor(
        "tail_drain_dummy", [1, 1], F32
    ).ap()

    x = nc.dram_tensor("x", [T, E], F32, kind="ExternalInput").ap()
    cst = nc.dram_tensor("cst", [128, CW], F32, kind="ExternalInput").ap()
    masks = nc.dram_tensor("masks", [128, MW], F32, kind="ExternalInput").ap()
    out = nc.dram_tensor("out", [T // 2, H], F32, kind="ExternalOutput").ap()

    with SplitDrainTileContext(nc) as tc:
        _build_body(nc, tc, x, cst, masks, out)
    split_multi_waits(nc)
    return nc


def _build_body(nc, tc, x, cst, masks, out):
    from contextlib import ExitStack

    ctx = ExitStack()
    with ctx:
        const_pool = ctx.enter_context(tc.tile_pool(name="const", bufs=1))
        persist = ctx.enter_context(tc.tile_pool(name="persist", bufs=1))
        xg_pool = ctx.enter_context(tc.tile_pool(name="xg", bufs=12))
        xt_pool = ctx.enter_context(tc.tile_pool(name="xt", bufs=16))
        exps_pool = ctx.enter_context(tc.tile_pool(name="exps", bufs=3))
        small_pool = ctx.enter_context(tc.tile_pool(name="small", bufs=4))

        # ---- constants; identity + first X tiles first so the PE can start
        id_sbr = const_pool.tile([128, 128], F32R)
        nc.gpsimd.dma_start(id_sbr[:], cst[:, C_ID : C_ID + 128])

        xgs = {}
        def load_group(g):
            tiles = []
            for a in range(4):
                xga = xg_pool.tile([128, E], F32R, tag="xga")
                nc.gpsimd.dma_start(
                    xga[:], x[g * GR + a * 128 : g * GR + (a + 1) * 128, :]
                )
                tiles.append(xga)
            xgs[g] = tiles

        load_group(0)
        cst_sb = const_pool.tile([128, CW], F32)
        w_sb = const_pool.tile([128, 3 * 512], F32R)  # wk|wq|wv images
        wkv_sb = const_pool.tile([128, NEC * 128], F32R)  # fused [Wk|Wv]
        nc.gpsimd.dma_start(cst_sb[:], cst[:])
        nc.gpsimd.dma_start(w_sb[:], cst[:, C_WK : C_WK + 3 * 512])
        nc.gpsimd.dma_start(wkv_sb[:], cst[:, C_WKV : C_WKV + NEC * 128])
        load_group(1)
        load_group(2)
        masks_sb = const_pool.tile([128, MW], F32)
        nc.gpsimd.dma_start(masks_sb[:], masks[:])

        id_sb = cst_sb[:, C_ID : C_ID + 128]
        expbias = cst_sb[:, C_BIAS : C_BIAS + 1]

        def wchunk(base, ec):  # [128, 64] fp32r weight chunk
            return w_sb[:, base + ec * H : base + (ec + 1) * H]

        def wkv(ec):  # [128, 128] fused [Wk | Wv] stationary
            return wkv_sb[:, ec * 128 : (ec + 1) * 128]

        # ---- persistent intermediates
        # kt_all: paired layout for PE row-tiling — K^T block i lives at
        # rows (i%2)*64..+64, cols (i//2)*128..+128
        kt_all = persist.tile([128, T // 2], F32R)
        # qt_all: rows 0:64 = Q^T (own queries), rows 64:128 = duplicate
        qt_all = persist.tile([128, T // 2], F32R)
        vaug = persist.tile([128, NQB * HA], F32R)   # V blocks + denom col
        outstage = persist.tile([128, 16 * H], F32)

        # denominator column = 8.0 (folds the /sqrt(64))
        nc.vector.tensor_copy(
            vaug[:].rearrange("p (i c) -> p i c", c=HA)[:, :, H],
            cst_sb[:, C_EIGHT : C_EIGHT + 32],
        )

        # ================= phase P: projections =================
        pp = ExitStack()
        with pp:
            xt_ps_pool = pp.enter_context(
                tc.tile_pool(name="xt_ps", bufs=4, space="PSUM")
            )
            kt_ps_pool = pp.enter_context(
                tc.tile_pool(name="kt_ps", bufs=1, space="PSUM")
            )
            qt_ps_pool = pp.enter_context(
                tc.tile_pool(name="qt_ps", bufs=1, space="PSUM")
            )
            vtr_ps_pool = pp.enter_context(
                tc.tile_pool(name="vtr_ps", bufs=2, space="PSUM")
            )

            for g in range(NG):
                if g + 3 < NG:
                    load_group(g + 3)
                xga = xgs.pop(g)

                # transpose X group: xt[ec] = X[group rows, ec-chunk].T
                xts = []
                for ec in range(NEC):
                    xt_ps = xt_ps_pool.tile([128, GR], F32R)
                    for a in range(4):
                        nc.tensor.transpose(
                            xt_ps[:, a * 128 : (a + 1) * 128],
                            xga[a][:, ec * 128 : (ec + 1) * 128],
                            id_sbr[:],
                        )
                    xt_sb = xt_pool.tile([128, GR], F32R)
                    eng = nc.vector.tensor_copy if ec % 2 == 0 else nc.scalar.copy
                    eng(xt_sb[:], xt_ps[:])
                    xts.append(xt_sb)

                # fused [Wk|Wv] chain: rows 0:64 = K^T, rows 64:128 = V^T
                kv_ps = kt_ps_pool.tile([128, GR], F32, tag="kv_ps")
                for ec in range(NEC):
                    nc.tensor.matmul(
                        kv_ps[:], wkv(ec), xts[ec][:],
                        start=(ec == 0), stop=(ec == NEC - 1),
                    )
                evens = kv_ps[0:64, :].rearrange("p (a r m) -> p r a m", a=2, r=2)
                nc.vector.tensor_copy(
                    kt_all[0:64, 2 * g * 128 : (2 * g + 2) * 128].rearrange(
                        "p (a m) -> p a m", a=2
                    ),
                    evens[:, 0, :, :],
                )
                ktodd = small_pool.tile([64, 256], F32R, tag="ktodd")
                nc.vector.tensor_copy(
                    ktodd[:].rearrange("p (a m) -> p a m", a=2),
                    evens[:, 1, :, :],
                )
                nc.gpsimd.dma_start(
                    kt_all[64:128, 2 * g * 128 : (2 * g + 2) * 128], ktodd[:]
                )

                # V^T (rows 64:128) -> V natural blocks via PE transpose
                vt_sb = small_pool.tile([128, GR], F32, tag="vt_sb")
                nc.scalar.copy(vt_sb[64:128, :], kv_ps[64:128, :])
                for a in range(4):
                    vtr_ps = vtr_ps_pool.tile([128, H], F32)
                    nc.tensor.transpose(
                        vtr_ps[:],
                        vt_sb[64:128, a * 128 : (a + 1) * 128],
                        id_sb[64:128, 64:128],
                    )
                    i = 4 * g + a
                    nc.vector.tensor_copy(
                        vaug[:, i * HA : i * HA + H], vtr_ps[:]
                    )

                # Q^T chain: own queries are local blocks 4g, 4g+2
                qt_ps = qt_ps_pool.tile([64, 256], F32)
                for ec in range(NEC):
                    rhs = xts[ec][:].rearrange(
                        "p (a r m) -> p r a m", a=2, r=2
                    )[:, 0, :, :]
                    nc.tensor.matmul(
                        qt_ps[:].rearrange("p (a m) -> p a m", a=2),
                        wchunk(512, ec),
                        rhs,
                        start=(ec == 0), stop=(ec == NEC - 1),
                    )
                nc.vector.tensor_copy(
                    qt_all[0:64, g * 256 : (g + 1) * 256], qt_ps[:]
                )
                if g % 2 == 1:  # duplicate finished 512-col pair to rows 64+
                    jj = g // 2
                    nc.gpsimd.dma_start(
                        qt_all[64:128, jj * 512 : (jj + 1) * 512],
                        qt_all[0:64, jj * 512 : (jj + 1) * 512],
                    )

        # ================= phase A: attention =================
        CH = 3  # kblocks per PSUM chunk (3 banks; double-buffered)
        pa = ExitStack()
        with pa:
            s_ps_pool = pa.enter_context(
                tc.tile_pool(name="s_ps", bufs=2, space="PSUM")
            )
            av_ps_pool = pa.enter_context(
                tc.tile_pool(name="av_ps", bufs=1, space="PSUM")
            )
            ot_ps_pool = pa.enter_context(
                tc.tile_pool(name="ot_ps", bufs=1, space="PSUM")
            )

            for jj in range(NMAC):
                # key blocks: 8jj full, then d=0..6 (local 8jj..8jj+6), wrap
                kblocks = list(range(8 * jj + 7)) + [31]
                nk = len(kblocks)  # 8jj+8
                qs = qt_all[:, jj * 512 : (jj + 1) * 512]

                av_ps = av_ps_pool.tile([HA, 512], F32)
                state = {"first": True}
                pend = None

                def issue_av(pend_, stop):
                    pexps, pblocks = pend_
                    for ci, i in enumerate(pblocks):
                        nc.tensor.matmul(
                            av_ps[:],
                            vaug[:, i * HA : (i + 1) * HA],
                            pexps[:, ci * 512 : (ci + 1) * 512],
                            start=state["first"],
                            stop=stop and (ci == len(pblocks) - 1),
                        )
                        state["first"] = False

                chunks = [kblocks[c : c + CH] for c in range(0, nk, CH)]
                for c, blocks in enumerate(chunks):
                    n = len(blocks) * 512
                    s_ps = s_ps_pool.tile([128, CH * 512], F32)
                    for ci, i in enumerate(blocks):
                        rg = (i % 2) * 64  # paired row group
                        nc.tensor.matmul(
                            s_ps[:, ci * 512 : (ci + 1) * 512],
                            kt_all[rg : rg + 64, (i // 2) * 128 : (i // 2 + 1) * 128],
                            qs[rg : rg + 64, :],
                            start=True,
                            stop=True,
                        )
                    exps = exps_pool.tile([128, CH * 512], F32R)
                    nc.scalar.activation(
                        exps[:, 0:n], s_ps[:, 0:n],
                        mybir.ActivationFunctionType.Exp,
                        bias=expbias,
                    )
                    # causal masks (0/1 multiplies on the sub-regions that
                    # are not all-ones)
                    for ci, i in enumerate(blocks):
                        d = i - 8 * jj
                        if i == 31:
                            w, mbase = 512, 7 * 512      # wrap block
                        elif 0 <= d <= 6:
                            w, mbase = (d // 2 + 1) * 128, d * 512
                        else:
                            continue
                        nc.vector.tensor_mul(
                            exps[:, ci * 512 : ci * 512 + w],
                            exps[:, ci * 512 : ci * 512 + w],
                            masks_sb[:, mbase : mbase + w],
                        )
                    if pend is not None:
                        issue_av(pend, stop=False)
                    pend = (exps, blocks)
                issue_av(pend, stop=True)

                # out^T [65, 512] -> transpose -> divide -> stage -> DMA out
                avs = small_pool.tile([HA, 512], F32, tag="avs")
                nc.scalar.copy(avs[:], av_ps[:])
                for t in range(4):
                    ot_ps = ot_ps_pool.tile([128, HA], F32)
                    nc.tensor.transpose(
                        ot_ps[:],
                        avs[:, t * 128 : (t + 1) * 128],
                        id_sb[0:HA, 0:HA],
                    )
                    rcp = small_pool.tile([128, 1], F32, tag="rcp")
                    nc.vector.reciprocal(rcp[:], ot_ps[:, H : H + 1])
                    q128 = 4 * jj + t
                    nc.vector.tensor_scalar_mul(
                        outstage[:, q128 * H : (q128 + 1) * H],
                        ot_ps[:, 0:H],
                        rcp[:],
                    )
                nc.gpsimd.dma_start(
                    out[jj * 512 : (jj + 1) * 512, :].rearrange(
                        "(b p) h -> p b h", p=128
                    ),
                    outstage[:, jj * 4 * H : (jj + 1) * 4 * H].rearrange(
                        "p (b h) -> p b h", h=H
                    ),
                )


def _host_inputs(input, Wq, Wk, Wv):
    """Build the 8 per-core input maps from the full problem inputs."""
    triu = np.triu(np.ones((128, 128), dtype=np.float32))
    ones = np.ones((128, 128), dtype=np.float32)
    zeros = np.zeros((128, 128), dtype=np.float32)

    def wimg(w):
        return (
            np.asarray(w, dtype=np.float32)
            .reshape(NEC, 128, H)
            .transpose(1, 0, 2)
            .reshape(128, NEC * H)
        )

    cst = np.empty((128, CW), dtype=np.float32)
    cst[:, C_ID : C_ID + 128] = np.eye(128, dtype=np.float32)
    cst[:, C_WK : C_WK + 512] = wimg(Wk)
    cst[:, C_WQ : C_WQ + 512] = wimg(Wq)
    cst[:, C_WV : C_WV + 512] = wimg(Wv)
    cst[:, C_EIGHT : C_EIGHT + 32] = 8.0
    cst[:, C_BIAS] = EXP_BIAS
    wk_i, wv_i = wimg(Wk), wimg(Wv)
    for ec in range(NEC):
        cst[:, C_WKV + ec * 128 : C_WKV + ec * 128 + 64] = wk_i[
            :, ec * 64 : (ec + 1) * 64
        ]
        cst[:, C_WKV + ec * 128 + 64 : C_WKV + (ec + 1) * 128] = wv_i[
            :, ec * 64 : (ec + 1) * 64
        ]

    in_maps = []
    for c in range(NCORES):
        b, s = divmod(c, 2)
        xb = np.asarray(input[b])
        x_rot = np.roll(xb, -128 * s, axis=0) if s else xb
        wrap = ones if s == 1 else zeros
        m = np.empty((128, MW), dtype=np.float32)
        for d in range(7):  # mask_d: 4 col-blocks vs rel = d - 2m
            for mcol in range(4):
                rel = d - 2 * mcol
                blk = ones if rel < 0 else (triu if rel == 0 else zeros)
                m[:, d * 512 + mcol * 128 : d * 512 + (mcol + 1) * 128] = blk
        m[:, 7 * 512 :] = np.tile(wrap, (1, 4))
        in_maps.append(
            {
                "x": np.ascontiguousarray(x_rot, dtype=np.float32),
                "cst": cst,
                "masks": m,
            }
        )
    return in_maps


def _assemble(results):
    """Scatter per-core striped outputs back to [B, T, H]."""
    out = np.empty((B, T, H), dtype=np.float32)
    for c in range(NCORES):
        b, s = divmod(c, 2)
        o = results[c]["out"].reshape(16, 128, H)  # own blocks, in order
        view = out[b].reshape(32, 128, H)
        view[s::2] = o
    return out


def kernel(input, Wq, Wk, Wv):
    from concourse.bass_utils import run_bass_kernel_spmd

    if "nc" not in _cache:
        _cache["nc"] = build_kernel()
    nc = _cache["nc"]
    in_maps = _host_inputs(input, Wq, Wk, Wv)
    res = run_bass_kernel_spmd(nc, in_maps, core_ids=list(range(NCORES)))
    return _assemble(res.results)
